# revision 3
# baseline (speedup 1.0000x reference)
"""Trainium2 Bass kernel v3 for MarginKLDivLoss-ColBERT (retrieval maxsim + KL).

Data-parallel over batch B=128 across 8 NeuronCores (16 examples/core).
Host does layout prep only (transpose/concat/dtype cast); all FLOPs on device.

v3 over v2: bf16 doc pipeline end-to-end (halves DMA and doubles the DVE
mask-multiply via the 2x 16-bit mode), mask replicas built by the idle Pool
engine with partition-broadcast into SBUF (so the multiply reads SBUF-only;
3 of 16 examples keep PE-matmul replicas to balance Pool), S computed
transposed in bf16 (4x fewer PE rows), token-chunk max as one strided DVE
reduce straight out of PSUM, token-partition max on Pool, one [16,*] epilogue.
"""

import os
import sys
from contextlib import ExitStack

sys.path.insert(0, "/opt/trn_rl_repo")

import numpy as np

import concourse.bass as bass  # noqa: F401
import concourse.bacc as bacc
import concourse.bass_isa as bass_isa
import concourse.mybir as mybir
import concourse.tile as tile
from concourse.bass_utils import run_bass_kernel_spmd

N_CORES = 8
B, Lq, Ld, D, N = 128, 32, 512, 128, 8
PB = B // N_CORES  # 16 examples per core
NDOC = N + 1  # pos + 8 negs
W = NDOC * Ld  # 4608 doc tokens per example

_f32 = mybir.dt.float32
_bf16 = mybir.dt.bfloat16
AF = mybir.ActivationFunctionType
ALU = mybir.AluOpType
AX = mybir.AxisListType

# examples whose mask replica comes from a PE matmul instead of the Pool
# broadcast (keeps Pool under the DVE/ACT ceiling)
PE_REPLICA = {3, 7, 11, 15}  # replicas via PE matmul; other 12 masks arrive pre-expanded

_PROGRAM = None
LAST_RESULTS = None


def _emit(ctx, tc, nc, aps):
    const = ctx.enter_context(tc.tile_pool(name="const", bufs=1))
    dpool = ctx.enter_context(tc.tile_pool(name="doc", bufs=3))
    mpool = ctx.enter_context(tc.tile_pool(name="masked", bufs=3))
    rpool = ctx.enter_context(tc.tile_pool(name="repb", bufs=4))
    spool = ctx.enter_context(tc.tile_pool(name="scratch", bufs=2))
    qpool = ctx.enter_context(tc.tile_pool(name="qs", bufs=3))
    stmp = ctx.enter_context(tc.tile_pool(name="stm", bufs=3))
    tiny = ctx.enter_context(tc.tile_pool(name="tiny", bufs=2))
    ps_q = ctx.enter_context(tc.tile_pool(name="ps_q", bufs=1, space="PSUM"))
    ps_rep = ctx.enter_context(tc.tile_pool(name="ps_rep", bufs=2, space="PSUM"))
    ps_st = ctx.enter_context(tc.tile_pool(name="ps_st", bufs=3, space="PSUM"))

    # ---- pin the one activation table (square+ln+exp+copy) up front ----
    from concourse.hw_specs import get_activation_tables

    tab_id = list(get_activation_tables(nc.m.arch)).index("natural_log_exp_and_others")
    nc.scalar.add_instruction(
        mybir.InstLoadActFuncSet(
            act_func_set_id=tab_id,
            name=nc.get_next_instruction_name(),
            ins=[],
            outs=[],
        )
    )

    # ---- constants / whole-core loads ----
    ones_bf = const.tile([65, D], _bf16)
    nc.gpsimd.memset(ones_bf[:], 1.0)
    onesf = const.tile([D, D], _f32)
    nc.gpsimd.memset(onesf[:], 1.0)
    ident = const.tile([Lq, Lq], _f32)
    nc.sync.dma_start(ident[:], aps["ident"][:])
    q_sb = const.tile([Lq, PB * D], _f32)
    nc.sync.dma_start(q_sb[:], aps["q_t"][:])
    # PE-replica mask rows: i-th PE example at partition 32*(i%3), block i//3
    maskp = const.tile([65, 2 * W], _bf16)
    for r in range(3):
        nc.sync.dma_start(maskp[32 * r : 32 * r + 1, :], aps["maskp"][r : r + 1, :])
    lab16 = const.tile([PB, N], _f32)
    nc.sync.dma_start(lab16[:], aps["labels"][:])
    svall = const.tile([1, PB * NDOC], _f32)
    sv16 = const.tile([PB, NDOC], _f32)

    # ---- q normalization (all 16 examples at once) ----
    qT_ps = ps_q.tile([D, PB * Lq], _f32, tag="qT_ps")
    for b in range(PB):
        nc.tensor.matmul(
            qT_ps[:, b * Lq : (b + 1) * Lq],
            q_sb[:, b * D : (b + 1) * D],
            ident[:],
            start=True,
            stop=True,
        )
    qT = const.tile([D, PB * Lq], _f32)
    nc.scalar.copy(qT[:], qT_ps[:])
    sq_q = spool.tile([D, PB * Lq], _f32, tag="sq_q")
    nc.scalar.activation(sq_q[:], qT[:], AF.Square)
    ssq_ps = ps_q.tile([D, PB * Lq], _f32, tag="ssq_ps")
    nc.tensor.matmul(
        ssq_ps[0:1, :],
        onesf[:, 0:1],
        sq_q[:],
        start=True,
        stop=True,
    )
    ssqc = tiny.tile([1, PB * Lq], _f32, tag="ssqc")
    nc.vector.tensor_scalar_max(ssqc[:], ssq_ps[0:1, :], 1e-24)
    lnq = tiny.tile([1, PB * Lq], _f32, tag="lnq")
    nc.scalar.activation(lnq[:], ssqc[:], AF.Ln)
    invq = tiny.tile([1, PB * Lq], _f32, tag="invq")
    nc.scalar.activation(invq[:], lnq[:], AF.Exp, scale=-0.5)
    repq_ps = ps_q.tile([D, PB * Lq], _f32, tag="repq_ps")
    nc.tensor.matmul(
        repq_ps[:],
        onesf[0:1, :],
        invq[:],
        start=True,
        stop=True,
    )
    qn = const.tile([D, PB * Lq], _f32)
    nc.vector.tensor_tensor(qn[:], qT[:], repq_ps[:], op=ALU.mult)

    # ---- main loop over examples (software-pipelined) ----
    # Pool broadcasts run LOOKAHEAD examples ahead so they never sit behind a
    # stalled all-reduce in Pool's in-order queue; the partition-max/sum tail
    # of example b-1 is emitted after example b's matmuls so no engine waits
    # on its own example's chain.
    LOOKAHEAD = 2
    xidx = {b: i for i, b in enumerate(bb for bb in range(PB) if bb not in PE_REPLICA)}

    def emit_broadcast(bb):
        repb = rpool.tile([D, W], _bf16, tag="repb")
        nc.sync.dma_start(repb[:], aps["maskx"][xidx[bb]])
        return repb

    repbs = {}
    for bb in range(LOOKAHEAD + 1):
        if bb < PB and bb not in PE_REPLICA:
            repbs[bb] = emit_broadcast(bb)

    def emit_tail(bb, m4):
        # max over the 128 token partitions on the Pool engine (SBUF only)
        mall = stmp.tile([D, NDOC * Lq], _bf16, tag="mall")
        nc.gpsimd.partition_all_reduce(
            mall[:], m4[:], channels=D, reduce_op=bass_isa.ReduceOp.max
        )
        # sum over queries -> per-doc maxsim scores for this example
        nc.vector.tensor_reduce(
            svall[0:1, bb * NDOC : (bb + 1) * NDOC].rearrange(
                "p (j o) -> p j o", j=NDOC, o=1
            ),
            mall[0:1, :].rearrange("p (j q) -> p j q", j=NDOC, q=Lq),
            axis=AX.X,
            op=ALU.add,
        )

    pending = None  # (b, m4) awaiting partition-max + query-sum
    for b in range(PB):
        dT = dpool.tile([D, W], _bf16, tag="dT")
        nc.sync.dma_start(dT[:], aps["docsT"][b])

        nb = b + LOOKAHEAD + 1
        if nb < PB and nb not in PE_REPLICA:
            repbs[nb] = emit_broadcast(nb)

        masked = mpool.tile([D, W], _bf16, tag="masked")
        if b in PE_REPLICA:
            i_pe = sorted(PE_REPLICA).index(b)
            base = 32 * (i_pe % 3)
            moff = (i_pe // 3) * W
            for j in range(NDOC):
                rep = ps_rep.tile([D, Ld], _f32, tag="rep")
                nc.tensor.matmul(
                    rep[:],
                    ones_bf[base : base + 1, :],
                    maskp[base : base + 1, moff + j * Ld : moff + (j + 1) * Ld],
                    start=True,
                    stop=True,
                )
                nc.vector.tensor_tensor(
                    masked[:, j * Ld : (j + 1) * Ld],
                    dT[:, j * Ld : (j + 1) * Ld],
                    rep[:],
                    op=ALU.mult,
                )
        else:
            nc.vector.tensor_tensor(masked[:], dT[:], repbs.pop(b)[:], op=ALU.mult)

        ssq9 = tiny.tile([D, NDOC], _f32, tag="ssq9")
        for j in range(NDOC):
            msq = spool.tile([D, Ld], _bf16, tag="msq")
            nc.scalar.activation(
                msq[:],
                masked[:, j * Ld : (j + 1) * Ld],
                AF.Square,
                accum_out=ssq9[:, j : j + 1],
            )

        ssq9c = tiny.tile([D, NDOC], _f32, tag="ssq9c")
        nc.vector.tensor_scalar_max(ssq9c[:], ssq9[:], 1e-24)
        ln9 = tiny.tile([D, NDOC], _f32, tag="ln9")
        nc.scalar.activation(ln9[:], ssq9c[:], AF.Ln)
        inv9 = tiny.tile([D, NDOC], _f32, tag="inv9")
        nc.scalar.activation(inv9[:], ln9[:], AF.Exp, scale=-0.5)

        qs = qpool.tile([D, NDOC * Lq], _bf16, tag="qs")
        nc.vector.tensor_tensor(
            qs[:].rearrange("p (j q) -> p j q", j=NDOC, q=Lq),
            qn[:, b * Lq : (b + 1) * Lq].unsqueeze(1).broadcast_to((D, NDOC, Lq)),
            inv9[:].unsqueeze(2).broadcast_to((D, NDOC, Lq)),
            op=ALU.mult,
        )

        # S transposed, bf16: token chunks on partitions; chunk-max straight
        # out of PSUM via one strided reduce per doc group
        m4 = stmp.tile([D, NDOC * Lq], _bf16, tag="m4")
        for g in range(3):  # doc groups of 4,4,1 packed per PSUM bank
            js = list(range(4 * g, min(NDOC, 4 * g + 4)))
            st_ps = ps_st.tile([D, 4 * D], _f32, tag="st_ps")
            for j in js:
                for c in range(4):
                    nc.tensor.matmul(
                        st_ps[:, (j - 4 * g) * D + c * Lq : (j - 4 * g) * D + (c + 1) * Lq],
                        masked[:, j * Ld + c * D : j * Ld + (c + 1) * D],
                        qs[:, j * Lq : (j + 1) * Lq],
                        start=True,
                        stop=True,
                    )
            nc.vector.tensor_reduce(
                m4[:, 4 * g * Lq : (4 * g + len(js)) * Lq].rearrange(
                    "p (d q) -> p d q", d=len(js), q=Lq
                ),
                st_ps[:, 0 : len(js) * D].rearrange(
                    "p (d c q) -> p d q c", d=len(js), c=4, q=Lq
                ),
                axis=AX.X,
                op=ALU.max,
            )
        if pending is not None:
            emit_tail(*pending)
        pending = (b, m4)
    emit_tail(*pending)

    # ---- epilogue on [16, *] ----
    nc.sync.dma_start(
        sv16[:], svall[0:1, :].rearrange("p (b j) -> p b j", b=PB, j=NDOC)
    )
    dsc = tiny.tile([PB, N], _f32, tag="dsc")
    nc.vector.tensor_scalar(
        dsc[:], sv16[:, 1:NDOC], sv16[:, 0:1], None, op0=ALU.subtract
    )
    mx = tiny.tile([PB, 1], _f32, tag="mx")
    nc.vector.tensor_reduce(mx[:], dsc[:], axis=AX.X, op=ALU.max)
    nmx = tiny.tile([PB, 1], _f32, tag="nmx")
    nc.vector.tensor_scalar_mul(nmx[:], mx[:], -1.0)
    e = tiny.tile([PB, N], _f32, tag="e")
    se = tiny.tile([PB, 1], _f32, tag="se")
    nc.scalar.activation(e[:], dsc[:], AF.Exp, bias=nmx[:], accum_out=se[:])
    lse0 = tiny.tile([PB, 1], _f32, tag="lse0")
    nc.scalar.activation(lse0[:], se[:], AF.Ln)
    lse = tiny.tile([PB, 1], _f32, tag="lse")
    nc.vector.tensor_tensor(lse[:], lse0[:], mx[:], op=ALU.add)
    elab = tiny.tile([PB, N], _f32, tag="elab")
    nc.scalar.activation(elab[:], lab16[:], AF.Exp)
    t1 = tiny.tile([PB, N], _f32, tag="t1")
    nc.vector.tensor_tensor(t1[:], lab16[:], dsc[:], op=ALU.subtract)
    t2 = tiny.tile([PB, N], _f32, tag="t2")
    nc.vector.tensor_scalar(t2[:], t1[:], lse[:], None, op0=ALU.add)
    t3 = tiny.tile([PB, N], _f32, tag="t3")
    nc.vector.tensor_tensor(t3[:], t2[:], elab[:], op=ALU.mult)
    out_sb = tiny.tile([PB, 1], _f32, tag="out_sb")
    nc.vector.tensor_reduce(out_sb[:], t3[:], axis=AX.X, op=ALU.add)
    nc.sync.dma_start(aps["out"][:], out_sb[:])


def build_program():
    nc = bacc.Bacc(
        "TRN2",
        target_bir_lowering=False,
        debug=False,
        enable_asserts=True,
        num_devices=N_CORES,
    )
    aps = {
        "q_t": nc.dram_tensor("q_t", [Lq, PB * D], _f32, kind="ExternalInput").ap(),
        "docsT": nc.dram_tensor("docsT", [PB, D, W], _bf16, kind="ExternalInput").ap(),
        "maskp": nc.dram_tensor("maskp", [3, 2 * W], _bf16, kind="ExternalInput").ap(),
        "maskx": nc.dram_tensor("maskx", [PB - 4, D, W], _bf16, kind="ExternalInput").ap(),
        "labels": nc.dram_tensor("labels", [PB, N], _f32, kind="ExternalInput").ap(),
        "ident": nc.dram_tensor("ident", [Lq, Lq], _f32, kind="ExternalInput").ap(),
        "out": nc.dram_tensor("out", [PB, 1], _f32, kind="ExternalOutput").ap(),
    }
    with tile.TileContext(nc) as tc:
        with ExitStack() as ctx:
            _emit(ctx, tc, nc, aps)
    nc.compile()
    return nc


def _np_bf16(x):
    import ml_dtypes

    return np.ascontiguousarray(x.astype(ml_dtypes.bfloat16))


def shard_inputs(q_emb, pos_emb, neg_emb, labels, pos_mask, neg_mask):
    q = np.ascontiguousarray(q_emb, dtype=np.float32)
    pos = np.asarray(pos_emb, dtype=np.float32)
    neg = np.asarray(neg_emb, dtype=np.float32)
    lab = np.ascontiguousarray(labels, dtype=np.float32)
    pm = np.asarray(pos_mask, dtype=np.float32)
    nm = np.asarray(neg_mask, dtype=np.float32)
    ident = np.eye(Lq, dtype=np.float32)
    in_maps = []
    for c in range(N_CORES):
        b0, b1 = c * PB, (c + 1) * PB
        docs = np.empty((PB, D, NDOC, Ld), np.float32)
        docs[:, :, 0, :] = pos[b0:b1].transpose(0, 2, 1)
        docs[:, :, 1:, :] = neg[:, b0:b1].transpose(1, 3, 0, 2)
        mrows = np.zeros((PB, NDOC, Ld), np.float32)
        for b in range(PB):
            mrows[b, 0] = pm[b0 + b]
            mrows[b, 1:] = nm[:, b0 + b]
        pe_rows = sorted(PE_REPLICA)
        x_rows = [b for b in range(PB) if b not in PE_REPLICA]
        mp = np.zeros((3, 2, NDOC, Ld), np.float32)
        for i, b in enumerate(pe_rows):
            mp[i % 3, i // 3] = mrows[b]
        mx = np.broadcast_to(
            mrows[x_rows].reshape(PB - 4, 1, W), (PB - 4, D, W)
        )
        in_maps.append(
            {
                "q_t": np.ascontiguousarray(
                    q[b0:b1].transpose(1, 0, 2).reshape(Lq, PB * D)
                ),
                "docsT": _np_bf16(docs.reshape(PB, D, W)),
                "maskp": _np_bf16(mp.reshape(3, 2 * W)),
                "maskx": _np_bf16(mx),
                "labels": np.ascontiguousarray(lab[b0:b1]),
                "ident": ident,
            }
        )
    return in_maps


def kernel(**inputs):
    global _PROGRAM, LAST_RESULTS
    if _PROGRAM is None:
        _PROGRAM = build_program()
    in_maps = shard_inputs(
        inputs["q_emb"],
        inputs["pos_emb"],
        inputs["neg_emb"],
        inputs["labels"],
        inputs["pos_mask"],
        inputs["neg_mask"],
    )
    trace = bool(int(os.environ.get("KBASS_TRACE", "0")))
    res = run_bass_kernel_spmd(_PROGRAM, in_maps, list(range(N_CORES)), trace=trace)
    LAST_RESULTS = res
    parts = np.concatenate(
        [np.asarray(res.results[c]["out"]).reshape(-1) for c in range(N_CORES)]
    )
    return np.float32(parts.sum(dtype=np.float64) / B)


# revision 4
# speedup vs baseline: 1.0684x; 1.0684x over previous
"""Trainium2 Bass kernel for MarginKLDivLoss-ColBERT (retrieval maxsim + KL).

Data-parallel over batch B=128 across 8 NeuronCores (16 examples/core).
Host does layout prep only (transpose/concat/dtype cast/mask replication);
all FLOPs happen on device.

Per core: one activation table pinned for the whole program (inverse norms
via exp(-0.5*ln(ssq)) so square/ln/exp share it -> no table reloads); docs
stream as bf16 (half the HBM traffic); masks arrive pre-replicated across
partitions for 12 of 16 examples (straight into SBUF so the bf16 multiply
runs the 2x 16-bit DVE mode) and as compact rows expanded by PE matmuls for
the rest; per-doc sum-of-masked-squares accumulates on ACT (8 docs) and DVE
(1 doc) in parallel; maxsim scores are computed TRANSPOSED (token chunks on
partitions, 4 chunked bf16 matmuls per doc -> 4x fewer PE rows), chunk-max
is one strided DVE reduce straight out of PSUM, token-partition max runs on
the Pool engine, and the KL epilogue is a single [16,*] batch of ops for
all examples. Output: per-example KL sums; host sums 128 values / B.
"""

import os
import sys
from contextlib import ExitStack

sys.path.insert(0, "/opt/trn_rl_repo")

import numpy as np

import concourse.bass as bass  # noqa: F401
import concourse.bacc as bacc
import concourse.bass_isa as bass_isa
import concourse.mybir as mybir
import concourse.tile as tile
from concourse.bass_utils import run_bass_kernel_spmd

N_CORES = 8
B, Lq, Ld, D, N = 128, 32, 512, 128, 8
PB = B // N_CORES  # 16 examples per core
NDOC = N + 1  # pos + 8 negs
W = NDOC * Ld  # 4608 doc tokens per example

_f32 = mybir.dt.float32
_bf16 = mybir.dt.bfloat16
AF = mybir.ActivationFunctionType
ALU = mybir.AluOpType
AX = mybir.AxisListType

# examples whose mask replica comes from a PE matmul instead of the Pool
# broadcast (keeps Pool under the DVE/ACT ceiling)
PE_REPLICA = {3, 7, 11, 15}  # replicas via PE matmul; other 12 masks arrive pre-expanded

_PROGRAM = None
LAST_RESULTS = None


def _emit(ctx, tc, nc, aps):
    const = ctx.enter_context(tc.tile_pool(name="const", bufs=1))
    dpool = ctx.enter_context(tc.tile_pool(name="doc", bufs=3))
    mpool = ctx.enter_context(tc.tile_pool(name="masked", bufs=3))
    rpool = ctx.enter_context(tc.tile_pool(name="repb", bufs=4))
    spool = ctx.enter_context(tc.tile_pool(name="scratch", bufs=2))
    qpool = ctx.enter_context(tc.tile_pool(name="qs", bufs=3))
    stmp = ctx.enter_context(tc.tile_pool(name="stm", bufs=3))
    tiny = ctx.enter_context(tc.tile_pool(name="tiny", bufs=2))
    ps_q = ctx.enter_context(tc.tile_pool(name="ps_q", bufs=1, space="PSUM"))
    ps_rep = ctx.enter_context(tc.tile_pool(name="ps_rep", bufs=2, space="PSUM"))
    ps_st = ctx.enter_context(tc.tile_pool(name="ps_st", bufs=3, space="PSUM"))

    # ---- pin the one activation table (square+ln+exp+copy) up front ----
    from concourse.hw_specs import get_activation_tables

    tab_id = list(get_activation_tables(nc.m.arch)).index("natural_log_exp_and_others")
    nc.scalar.add_instruction(
        mybir.InstLoadActFuncSet(
            act_func_set_id=tab_id,
            name=nc.get_next_instruction_name(),
            ins=[],
            outs=[],
        )
    )

    # ---- constants / whole-core loads ----
    ones_bf = const.tile([65, D], _bf16)
    nc.gpsimd.memset(ones_bf[:], 1.0)
    onesf = const.tile([D, D], _f32)
    nc.gpsimd.memset(onesf[:], 1.0)
    ident = const.tile([Lq, Lq], _f32)
    nc.sync.dma_start(ident[:], aps["ident"][:])
    q_sb = const.tile([Lq, PB * D], _f32)
    nc.sync.dma_start(q_sb[:], aps["q_t"][:])
    # PE-replica mask rows: i-th PE example at partition 32*(i%3), block i//3
    maskp = const.tile([65, 2 * W], _bf16)
    for r in range(3):
        nc.sync.dma_start(maskp[32 * r : 32 * r + 1, :], aps["maskp"][r : r + 1, :])
    lab16 = const.tile([PB, N], _f32)
    nc.sync.dma_start(lab16[:], aps["labels"][:])
    svall = const.tile([1, PB * NDOC], _f32)
    sv16 = const.tile([PB, NDOC], _f32)

    # ---- q normalization (all 16 examples at once) ----
    qT_ps = ps_q.tile([D, PB * Lq], _f32, tag="qT_ps")
    for b in range(PB):
        nc.tensor.matmul(
            qT_ps[:, b * Lq : (b + 1) * Lq],
            q_sb[:, b * D : (b + 1) * D],
            ident[:],
            start=True,
            stop=True,
        )
    qT = const.tile([D, PB * Lq], _f32)
    nc.scalar.copy(qT[:], qT_ps[:])
    sq_q = spool.tile([D, PB * Lq], _f32, tag="sq_q")
    nc.scalar.activation(sq_q[:], qT[:], AF.Square)
    ssq_ps = ps_q.tile([D, PB * Lq], _f32, tag="ssq_ps")
    nc.tensor.matmul(
        ssq_ps[0:1, :],
        onesf[:, 0:1],
        sq_q[:],
        start=True,
        stop=True,
    )
    ssqc = tiny.tile([1, PB * Lq], _f32, tag="ssqc")
    nc.vector.tensor_scalar_max(ssqc[:], ssq_ps[0:1, :], 1e-24)
    lnq = tiny.tile([1, PB * Lq], _f32, tag="lnq")
    nc.scalar.activation(lnq[:], ssqc[:], AF.Ln)
    invq = tiny.tile([1, PB * Lq], _f32, tag="invq")
    nc.scalar.activation(invq[:], lnq[:], AF.Exp, scale=-0.5)
    repq_ps = ps_q.tile([D, PB * Lq], _f32, tag="repq_ps")
    nc.tensor.matmul(
        repq_ps[:],
        onesf[0:1, :],
        invq[:],
        start=True,
        stop=True,
    )
    qn = const.tile([D, PB * Lq], _f32)
    nc.vector.tensor_tensor(qn[:], qT[:], repq_ps[:], op=ALU.mult)

    # ---- main loop over examples (software-pipelined) ----
    # Pool broadcasts run LOOKAHEAD examples ahead so they never sit behind a
    # stalled all-reduce in Pool's in-order queue; the partition-max/sum tail
    # of example b-1 is emitted after example b's matmuls so no engine waits
    # on its own example's chain.
    LOOKAHEAD = 2
    xidx = {b: i for i, b in enumerate(bb for bb in range(PB) if bb not in PE_REPLICA)}

    def emit_broadcast(bb):
        repb = rpool.tile([D, W], _bf16, tag="repb")
        nc.sync.dma_start(repb[:], aps["maskx"][xidx[bb]])
        return repb

    repbs = {}
    for bb in range(LOOKAHEAD + 1):
        if bb < PB and bb not in PE_REPLICA:
            repbs[bb] = emit_broadcast(bb)

    def emit_tail(bb, m4):
        # max over the 128 token partitions on the Pool engine (SBUF only)
        mall = stmp.tile([D, NDOC * Lq], _bf16, tag="mall")
        nc.gpsimd.partition_all_reduce(
            mall[:], m4[:], channels=D, reduce_op=bass_isa.ReduceOp.max
        )
        # sum over queries -> per-doc maxsim scores for this example
        nc.vector.tensor_reduce(
            svall[0:1, bb * NDOC : (bb + 1) * NDOC].rearrange(
                "p (j o) -> p j o", j=NDOC, o=1
            ),
            mall[0:1, :].rearrange("p (j q) -> p j q", j=NDOC, q=Lq),
            axis=AX.X,
            op=ALU.add,
        )

    pending = None  # (b, m4) awaiting partition-max + query-sum
    for b in range(PB):
        dT = dpool.tile([D, W], _bf16, tag="dT")
        nc.sync.dma_start(dT[:], aps["docsT"][b])

        nb = b + LOOKAHEAD + 1
        if nb < PB and nb not in PE_REPLICA:
            repbs[nb] = emit_broadcast(nb)

        masked = mpool.tile([D, W], _bf16, tag="masked")
        if b in PE_REPLICA:
            i_pe = sorted(PE_REPLICA).index(b)
            base = 32 * (i_pe % 3)
            moff = (i_pe // 3) * W
            for j in range(NDOC):
                rep = ps_rep.tile([D, Ld], _f32, tag="rep")
                nc.tensor.matmul(
                    rep[:],
                    ones_bf[base : base + 1, :],
                    maskp[base : base + 1, moff + j * Ld : moff + (j + 1) * Ld],
                    start=True,
                    stop=True,
                )
                nc.vector.tensor_tensor(
                    masked[:, j * Ld : (j + 1) * Ld],
                    dT[:, j * Ld : (j + 1) * Ld],
                    rep[:],
                    op=ALU.mult,
                )
        else:
            nc.vector.tensor_tensor(masked[:], dT[:], repbs.pop(b)[:], op=ALU.mult)

        # ssq: doc 0 on DVE (bf16 multiply + reduce-add; masked*dT ==
        # mask*dT^2 exactly since docs are bf16, masks 0/1), docs 1-8 on ACT
        ssq9a = tiny.tile([D, 1], _f32, tag="ssq9a")
        ssq9b = tiny.tile([D, NDOC - 1], _f32, tag="ssq9b")
        msq0 = spool.tile([D, Ld], _bf16, tag="msq0")
        nc.vector.tensor_tensor(
            msq0[:], masked[:, 0:Ld], dT[:, 0:Ld], op=ALU.mult
        )
        nc.vector.tensor_reduce(ssq9a[:], msq0[:], axis=AX.X, op=ALU.add)
        for j in range(1, NDOC):
            msq = spool.tile([D, Ld], _bf16, tag="msq")
            nc.scalar.activation(
                msq[:],
                masked[:, j * Ld : (j + 1) * Ld],
                AF.Square,
                accum_out=ssq9b[:, j - 1 : j],
            )

        ssq9c = tiny.tile([D, NDOC], _f32, tag="ssq9c")
        nc.vector.tensor_scalar_max(ssq9c[:, 0:1], ssq9a[:], 1e-24)
        nc.vector.tensor_scalar_max(ssq9c[:, 1:NDOC], ssq9b[:], 1e-24)
        ln9 = tiny.tile([D, NDOC], _f32, tag="ln9")
        nc.scalar.activation(ln9[:], ssq9c[:], AF.Ln)
        inv9 = tiny.tile([D, NDOC], _f32, tag="inv9")
        nc.scalar.activation(inv9[:], ln9[:], AF.Exp, scale=-0.5)

        qs = qpool.tile([D, NDOC * Lq], _bf16, tag="qs")
        nc.vector.tensor_tensor(
            qs[:].rearrange("p (j q) -> p j q", j=NDOC, q=Lq),
            qn[:, b * Lq : (b + 1) * Lq].unsqueeze(1).broadcast_to((D, NDOC, Lq)),
            inv9[:].unsqueeze(2).broadcast_to((D, NDOC, Lq)),
            op=ALU.mult,
        )

        # S transposed, bf16: token chunks on partitions; chunk-max straight
        # out of PSUM via one strided reduce per doc group
        m4 = stmp.tile([D, NDOC * Lq], _bf16, tag="m4")
        for g in range(3):  # doc groups of 4,4,1 packed per PSUM bank
            js = list(range(4 * g, min(NDOC, 4 * g + 4)))
            st_ps = ps_st.tile([D, 4 * D], _f32, tag="st_ps")
            for j in js:
                for c in range(4):
                    nc.tensor.matmul(
                        st_ps[:, (j - 4 * g) * D + c * Lq : (j - 4 * g) * D + (c + 1) * Lq],
                        masked[:, j * Ld + c * D : j * Ld + (c + 1) * D],
                        qs[:, j * Lq : (j + 1) * Lq],
                        start=True,
                        stop=True,
                    )
            nc.vector.tensor_reduce(
                m4[:, 4 * g * Lq : (4 * g + len(js)) * Lq].rearrange(
                    "p (d q) -> p d q", d=len(js), q=Lq
                ),
                st_ps[:, 0 : len(js) * D].rearrange(
                    "p (d c q) -> p d q c", d=len(js), c=4, q=Lq
                ),
                axis=AX.X,
                op=ALU.max,
            )
        if pending is not None:
            emit_tail(*pending)
        pending = (b, m4)
    emit_tail(*pending)

    # ---- epilogue on [16, *] ----
    nc.sync.dma_start(
        sv16[:], svall[0:1, :].rearrange("p (b j) -> p b j", b=PB, j=NDOC)
    )
    dsc = tiny.tile([PB, N], _f32, tag="dsc")
    nc.vector.tensor_scalar(
        dsc[:], sv16[:, 1:NDOC], sv16[:, 0:1], None, op0=ALU.subtract
    )
    mx = tiny.tile([PB, 1], _f32, tag="mx")
    nc.vector.tensor_reduce(mx[:], dsc[:], axis=AX.X, op=ALU.max)
    nmx = tiny.tile([PB, 1], _f32, tag="nmx")
    nc.vector.tensor_scalar_mul(nmx[:], mx[:], -1.0)
    e = tiny.tile([PB, N], _f32, tag="e")
    se = tiny.tile([PB, 1], _f32, tag="se")
    nc.scalar.activation(e[:], dsc[:], AF.Exp, bias=nmx[:], accum_out=se[:])
    lse0 = tiny.tile([PB, 1], _f32, tag="lse0")
    nc.scalar.activation(lse0[:], se[:], AF.Ln)
    lse = tiny.tile([PB, 1], _f32, tag="lse")
    nc.vector.tensor_tensor(lse[:], lse0[:], mx[:], op=ALU.add)
    elab = tiny.tile([PB, N], _f32, tag="elab")
    nc.scalar.activation(elab[:], lab16[:], AF.Exp)
    t1 = tiny.tile([PB, N], _f32, tag="t1")
    nc.vector.tensor_tensor(t1[:], lab16[:], dsc[:], op=ALU.subtract)
    t2 = tiny.tile([PB, N], _f32, tag="t2")
    nc.vector.tensor_scalar(t2[:], t1[:], lse[:], None, op0=ALU.add)
    t3 = tiny.tile([PB, N], _f32, tag="t3")
    nc.vector.tensor_tensor(t3[:], t2[:], elab[:], op=ALU.mult)
    out_sb = tiny.tile([PB, 1], _f32, tag="out_sb")
    nc.vector.tensor_reduce(out_sb[:], t3[:], axis=AX.X, op=ALU.add)
    nc.sync.dma_start(aps["out"][:], out_sb[:])


def build_program():
    nc = bacc.Bacc(
        "TRN2",
        target_bir_lowering=False,
        debug=False,
        enable_asserts=True,
        num_devices=N_CORES,
    )
    aps = {
        "q_t": nc.dram_tensor("q_t", [Lq, PB * D], _f32, kind="ExternalInput").ap(),
        "docsT": nc.dram_tensor("docsT", [PB, D, W], _bf16, kind="ExternalInput").ap(),
        "maskp": nc.dram_tensor("maskp", [3, 2 * W], _bf16, kind="ExternalInput").ap(),
        "maskx": nc.dram_tensor("maskx", [PB - 4, D, W], _bf16, kind="ExternalInput").ap(),
        "labels": nc.dram_tensor("labels", [PB, N], _f32, kind="ExternalInput").ap(),
        "ident": nc.dram_tensor("ident", [Lq, Lq], _f32, kind="ExternalInput").ap(),
        "out": nc.dram_tensor("out", [PB, 1], _f32, kind="ExternalOutput").ap(),
    }
    with tile.TileContext(nc) as tc:
        with ExitStack() as ctx:
            _emit(ctx, tc, nc, aps)
    nc.compile()
    return nc


def _np_bf16(x):
    import ml_dtypes

    return np.ascontiguousarray(x.astype(ml_dtypes.bfloat16))


def shard_inputs(q_emb, pos_emb, neg_emb, labels, pos_mask, neg_mask):
    q = np.ascontiguousarray(q_emb, dtype=np.float32)
    pos = np.asarray(pos_emb, dtype=np.float32)
    neg = np.asarray(neg_emb, dtype=np.float32)
    lab = np.ascontiguousarray(labels, dtype=np.float32)
    pm = np.asarray(pos_mask, dtype=np.float32)
    nm = np.asarray(neg_mask, dtype=np.float32)
    ident = np.eye(Lq, dtype=np.float32)
    in_maps = []
    for c in range(N_CORES):
        b0, b1 = c * PB, (c + 1) * PB
        docs = np.empty((PB, D, NDOC, Ld), np.float32)
        docs[:, :, 0, :] = pos[b0:b1].transpose(0, 2, 1)
        docs[:, :, 1:, :] = neg[:, b0:b1].transpose(1, 3, 0, 2)
        mrows = np.zeros((PB, NDOC, Ld), np.float32)
        for b in range(PB):
            mrows[b, 0] = pm[b0 + b]
            mrows[b, 1:] = nm[:, b0 + b]
        pe_rows = sorted(PE_REPLICA)
        x_rows = [b for b in range(PB) if b not in PE_REPLICA]
        mp = np.zeros((3, 2, NDOC, Ld), np.float32)
        for i, b in enumerate(pe_rows):
            mp[i % 3, i // 3] = mrows[b]
        mx = np.broadcast_to(
            mrows[x_rows].reshape(PB - 4, 1, W), (PB - 4, D, W)
        )
        in_maps.append(
            {
                "q_t": np.ascontiguousarray(
                    q[b0:b1].transpose(1, 0, 2).reshape(Lq, PB * D)
                ),
                "docsT": _np_bf16(docs.reshape(PB, D, W)),
                "maskp": _np_bf16(mp.reshape(3, 2 * W)),
                "maskx": _np_bf16(mx),
                "labels": np.ascontiguousarray(lab[b0:b1]),
                "ident": ident,
            }
        )
    return in_maps


def kernel(**inputs):
    global _PROGRAM, LAST_RESULTS
    if _PROGRAM is None:
        _PROGRAM = build_program()
    in_maps = shard_inputs(
        inputs["q_emb"],
        inputs["pos_emb"],
        inputs["neg_emb"],
        inputs["labels"],
        inputs["pos_mask"],
        inputs["neg_mask"],
    )
    trace = bool(int(os.environ.get("KBASS_TRACE", "0")))
    res = run_bass_kernel_spmd(_PROGRAM, in_maps, list(range(N_CORES)), trace=trace)
    LAST_RESULTS = res
    parts = np.concatenate(
        [np.asarray(res.results[c]["out"]).reshape(-1) for c in range(N_CORES)]
    )
    return np.float32(parts.sum(dtype=np.float64) / B)
